# revision 13
# baseline (speedup 1.0000x reference)
"""Trainium2 Bass kernel for nn_BasicBlock (MoE-combined residual conv block).

  out = relu(bn2(conv3x3(relu(bn1(conv3x3(x, w1e))), w2e)) + x)
  w{1,2}e = sum_e alpha[e] * w{1,2}[e]   (host-side: linear in weights)

Strategy (per NeuronCore, data-parallel over batch: 32 imgs -> 4 per core x 8,
processed as 2 image PAIRS):

  - Every SBUF plane tile is image-PAIRED: [128, 114*114] fp16 with
    partitions 0-63 = padded plane of the even image and 64-127 = odd image.
    No shifted copies anywhere.
  - Both convs use block-diagonal lhsT per tap (t,d): [128,128] fp16 holding
    the same 64x64 weight block twice, so ONE matmul per tap processes both
    images: 9 matmuls per 4-row output chunk per image pair (4.5 per image,
    vs 6 for the dual-row-shift scheme on single images).  fp16 matmuls
    stream 1 column/cycle at 2.4 GHz; accumulation is fp32 in PSUM.
  - conv1 eviction: ONE ACT op per pair-chunk (relu + bn1 bias for both
    image halves) writing the paired mid tile.
  - conv2 eviction: ONE DVE tensor_add per pair-chunk applies the residual
    straight out of PSUM (fp16 x pair tile), ONE ACT op applies bn2 bias +
    relu, then 2 DMAs store fp32 to HBM.
  - x is loaded in fp32 row bands for both images at once ([128, band*W]
    staging tile) and cast to fp16 on DVE; bands keep the first conv group
    off the critical path for pair 0.

PE column count: 2 convs * 9 taps * 28 chunks * 448 cols * 2 pairs = 452k
columns (~188 us streaming) vs 602k for the 6-stream-per-image baseline.
Engine loads stay ~<40%: ACT ~70us, DVE ~55us, Pool ~10us.
"""

import numpy as np

import concourse.mybir as mybir
import concourse.tile as tile
from concourse import bacc
from concourse.bass_utils import run_bass_kernel_spmd

F32 = mybir.dt.float32
F16 = mybir.dt.float16
AF = mybir.ActivationFunctionType
ALU = mybir.AluOpType

EPS = 1e-5
N_CORES = 8
C = 64   # channels (in == out)
R = 4    # output rows per PSUM chunk
BAND = 16  # x load/cast band rows


def build_nc(B, H, W):
    """Bass program: B images of [64, H, W] per core (B even)."""
    Hp, Wp = H + 2, W + 2
    N = R * W                     # psum free size per chunk
    nwin = H // R
    assert H % R == 0 and B % 2 == 0
    band = BAND if H % BAND == 0 else H
    nbands = H // band

    nc = bacc.Bacc("TRN2", target_bir_lowering=False, debug=False,
                   enable_asserts=False, num_devices=N_CORES)

    xin = nc.dram_tensor("xin", [B, C, H, W], F32, kind="ExternalInput").ap()
    w1_d = nc.dram_tensor("w1d", [128, 9 * 128], F16, kind="ExternalInput").ap()
    w2_d = nc.dram_tensor("w2d", [128, 9 * 128], F16, kind="ExternalInput").ap()
    b1_d = nc.dram_tensor("b1", [128, 1], F32, kind="ExternalInput").ap()
    b2_d = nc.dram_tensor("b2", [128, 1], F32, kind="ExternalInput").ap()
    yout = nc.dram_tensor("yout", [B, C, H, W], F32, kind="ExternalOutput").ap()

    with tile.TileContext(nc) as tc:
        with (
            tc.tile_pool(name="wpool", bufs=1) as wpool,
            tc.tile_pool(name="xpool", bufs=2) as xpool,
            tc.tile_pool(name="fpool", bufs=3) as fpool,
            tc.tile_pool(name="mpool", bufs=2) as mpool,
            tc.tile_pool(name="pspool", bufs=8, space="PSUM") as pspool,
            tc.tile_pool(name="u2pool", bufs=4) as u2pool,
            tc.tile_pool(name="opool", bufs=4) as opool,
        ):
            w1t = wpool.tile([128, 9 * 128], F16)
            w2t = wpool.tile([128, 9 * 128], F16)
            b1t = wpool.tile([128, 1], F32)
            b2t = wpool.tile([128, 1], F32)
            # weight/bias loads go on the (idle) scalar queue so the sync
            # queue can issue the first x bands immediately
            nc.scalar.dma_start(w1t[:, :], w1_d[:, :])
            nc.scalar.dma_start(w2t[:, :], w2_d[:, :])
            nc.scalar.dma_start(b1t[:, :], b1_d[:, :])
            nc.scalar.dma_start(b2t[:, :], b2_d[:, :])

            def x_prep(pair):
                """Load + cast both images of a pair into one padded tile."""
                a = 2 * pair
                xt = xpool.tile([128, Hp * Wp], F16, tag="xt",
                                name=f"xt_{pair}")
                xr = xt[:, :].rearrange("p (h w) -> p h w", w=Wp)
                # pair 0 is on the critical path: tiny first bands so the
                # first conv chunk is ready sooner, and its first band is
                # emitted BEFORE the border memsets so the first cast isn't
                # queued behind them
                if pair == 0 and band == 16 and H == 112:
                    sizes = [4, 4, 8, 8, 8, 16, 16, 16, 16, 16]
                else:
                    sizes = [band] * nbands

                def emit_band(b, bsz, r0):
                    xf = fpool.tile([128, band * W], F32, tag="xf",
                                    name=f"xf_{pair}_{b}")
                    xfr = xf[:, 0:bsz * W].rearrange("p (h w) -> p h w", w=W)
                    nc.sync.dma_start(xfr[0:64], xin[a][:, r0:r0 + bsz, :])
                    nc.sync.dma_start(xfr[64:128],
                                      xin[a + 1][:, r0:r0 + bsz, :])
                    nc.vector.tensor_copy(
                        xr[0:128, r0 + 1:r0 + bsz + 1, 1:W + 1], xfr)

                nhead = 2 if pair == 0 else 0
                r0 = 0
                for b in range(nhead):
                    emit_band(b, sizes[b], r0)
                    r0 += sizes[b]
                nc.gpsimd.memset(xr[:, 0, :], 0.0)
                nc.gpsimd.memset(xr[:, Hp - 1, :], 0.0)
                nc.gpsimd.memset(xr[:, :, 0], 0.0)
                nc.gpsimd.memset(xr[:, :, Wp - 1], 0.0)
                for b in range(nhead, len(sizes)):
                    emit_band(b, sizes[b], r0)
                    r0 += sizes[b]
                return xr

            def conv1(xr, mr):
                """conv1 + bn1 + relu into paired mid tile."""
                for cd in range(nwin):
                    c = cd * R
                    P = pspool.tile([128, 512], F32, tag="ps",
                                    name=f"ps1_{cd}")
                    ti = 0
                    for t in range(3):
                        for d in range(3):
                            nc.tensor.matmul(
                                P[:, 0:N],
                                lhsT=w1t[:, ti * 128:(ti + 1) * 128],
                                rhs=xr[0:128, c + t:c + t + R, d:d + W],
                                start=(ti == 0), stop=(ti == 8))
                            ti += 1
                    nc.scalar.activation(
                        mr[0:128, c + 1:c + 1 + R, 1:W + 1],
                        P[:, 0:N].rearrange("p (h w) -> p h w", w=W),
                        AF.Relu, bias=b1t[:, 0:1])

            def conv2(mr, xr, pair):
                """conv2 + bn2 + residual + relu -> HBM."""
                a = 2 * pair
                for cd in range(nwin):
                    c = cd * R
                    P = pspool.tile([128, 512], F32, tag="ps",
                                    name=f"ps2_{cd}")
                    ti = 0
                    for t in range(3):
                        for d in range(3):
                            nc.tensor.matmul(
                                P[:, 0:N],
                                lhsT=w2t[:, ti * 128:(ti + 1) * 128],
                                rhs=mr[0:128, c + t:c + t + R, d:d + W],
                                start=(ti == 0), stop=(ti == 8))
                            ti += 1
                    u2 = u2pool.tile([128, N], F32, tag="u2",
                                     name=f"u2_{pair}_{cd}")
                    nc.vector.tensor_add(
                        u2[:, :].rearrange("p (h w) -> p h w", w=W),
                        P[:, 0:N].rearrange("p (h w) -> p h w", w=W),
                        xr[0:128, c + 1:c + 1 + R, 1:W + 1])
                    o = opool.tile([128, N], F32, tag="o",
                                   name=f"o_{pair}_{cd}")
                    nc.scalar.activation(o[:, :], u2[:, :], AF.Relu,
                                         bias=b2t[:, 0:1])
                    nc.sync.dma_start(
                        yout[a:a + 2, :, c:c + R, :].rearrange(
                            "b ch h w -> (b ch) h w"),
                        o[:, :].rearrange("p (h w) -> p h w", w=W))

            xrs = {0: x_prep(0)}
            for pair in range(B // 2):
                xr = xrs.pop(pair)
                mt = mpool.tile([128, Hp * Wp], F16, tag="mt",
                                name=f"mt_{pair}")
                mr = mt[:, :].rearrange("p (h w) -> p h w", w=Wp)
                nc.gpsimd.memset(mr[:, 0, :], 0.0)
                nc.gpsimd.memset(mr[:, Hp - 1, :], 0.0)
                nc.gpsimd.memset(mr[:, :, 0], 0.0)
                nc.gpsimd.memset(mr[:, :, Wp - 1], 0.0)
                conv1(xr, mr)
                if pair + 1 < B // 2:
                    xrs[pair + 1] = x_prep(pair + 1)
                conv2(mr, xr, pair)
    nc.compile()
    return nc


def prepare_weights(w1, w2, alpha, bn1_gamma, bn1_beta, bn1_mean, bn1_var,
                    bn2_gamma, bn2_beta, bn2_mean, bn2_var):
    w1e = np.einsum('e,eoihw->oihw', alpha.astype(np.float64),
                    w1.astype(np.float64))
    w2e = np.einsum('e,eoihw->oihw', alpha.astype(np.float64),
                    w2.astype(np.float64))
    s1 = bn1_gamma / np.sqrt(bn1_var + EPS)
    b1 = bn1_beta - bn1_mean * s1
    s2 = bn2_gamma / np.sqrt(bn2_var + EPS)
    b2 = bn2_beta - bn2_mean * s2
    w1e = (w1e * s1[:, None, None, None]).astype(np.float16)  # fold bn1 scale
    w2e = (w2e * s2[:, None, None, None]).astype(np.float16)  # fold bn2 scale

    # block-diagonal lhsT per tap (t,d): same 64x64 block for both images
    def blockdiag(we):
        wd = np.zeros((128, 9 * 128), np.float16)
        for t in range(3):
            for d in range(3):
                ti = t * 3 + d
                wd[0:64, ti * 128:ti * 128 + 64] = we[:, :, t, d].T
                wd[64:128, ti * 128 + 64:ti * 128 + 128] = we[:, :, t, d].T
        return wd

    b1v = np.tile(b1.astype(np.float32), 2).reshape(128, 1)
    b2v = np.tile(b2.astype(np.float32), 2).reshape(128, 1)
    return blockdiag(w1e), blockdiag(w2e), b1v, b2v


_NC_CACHE = {}


def kernel(x, w1, w2, alpha,
           bn1_gamma, bn1_beta, bn1_mean, bn1_var,
           bn2_gamma, bn2_beta, bn2_mean, bn2_var):
    x = np.ascontiguousarray(np.asarray(x, dtype=np.float32))
    B_total, _, H, W = x.shape
    Bc = B_total // N_CORES
    w1d, w2d, b1v, b2v = prepare_weights(
        np.asarray(w1, np.float32), np.asarray(w2, np.float32),
        np.asarray(alpha, np.float32),
        np.asarray(bn1_gamma, np.float32), np.asarray(bn1_beta, np.float32),
        np.asarray(bn1_mean, np.float32), np.asarray(bn1_var, np.float32),
        np.asarray(bn2_gamma, np.float32), np.asarray(bn2_beta, np.float32),
        np.asarray(bn2_mean, np.float32), np.asarray(bn2_var, np.float32))

    key = (Bc, H, W)
    if key not in _NC_CACHE:
        _NC_CACHE[key] = build_nc(Bc, H, W)
    nc = _NC_CACHE[key]

    in_maps = []
    for cid in range(N_CORES):
        in_maps.append({
            "xin": x[cid * Bc:(cid + 1) * Bc],
            "w1d": w1d, "w2d": w2d, "b1": b1v, "b2": b2v,
        })
    res = run_bass_kernel_spmd(nc, in_maps, core_ids=list(range(N_CORES)))
    out = np.concatenate([res.results[cid]["yout"] for cid in range(N_CORES)],
                         axis=0)
    return out
